# revision 8
# baseline (speedup 1.0000x reference)
"""Sliding-window attention Trainium2 Bass kernel.

Problem: B=4, H=32, L=4096, D=128, window=512.
reference: attends over the LAST w=512 key/value positions; query row i may
only see window slot j when j <= i (slots are key positions L-w+j).

Sharding: B*H = 128 (b,h) pairs split across 8 cores -> 16 heads/core.
Pure data parallelism, no collectives.

Per-head on-device algorithm (matmuls in f32r, 1 cycle/row):
  S^T chunks [wc=128, qg=512] = (K^T chunk)^T . (Q^T group)    (PE -> PSUM)
  mask-add on the first 512 queries                            (DVE)
  P^T = exp(S^T * 1/sqrt(D))                                   (ACT, one op/group)
  rowsum[1, qg] = ones^T @ P^T chunks (accumulated)            (PE)
  copy rowsum into row g of a per-head [8, qg] tile            (ACT)
  O^T [D, qg] += V_c^T @ P^T chunks; copy to SBUF unnormalized (PE, DVE)
  per head: recip of all 8x512 sums in one DVE op (8 lanes)    (DVE)
  per group: partition-broadcast recip row, multiply in place  (GPSIMD, DVE)
Host pre-transposes Q/K to [head, D, L] layout and post-transposes the
[head, D, L] output back to [B, H, L, D].
"""

import math
from contextlib import ExitStack

import numpy as np

N_CORES = 8
B, H, L, D = 4, 32, 4096, 128
W = 512            # window
HEADS_PER_CORE = (B * H) // N_CORES   # 16
QG = 512           # queries per group
NG = L // QG       # groups per head (8)
NCHUNK = W // 128  # 4 window chunks
NEG = -1.0e9       # additive mask value (pre-scale)
SCALE = 1.0 / math.sqrt(D)

_COMPILED = None


def _build():
    import concourse.tile as tile
    from concourse import bacc, mybir

    nc = bacc.Bacc("TRN2", target_bir_lowering=False, debug=False,
                   num_devices=N_CORES)

    f32r = mybir.dt.float32r
    f32 = mybir.dt.float32

    qT = nc.dram_tensor("qT", [HEADS_PER_CORE, D, L], f32r, kind="ExternalInput").ap()
    kT = nc.dram_tensor("kT", [HEADS_PER_CORE, D, W], f32r, kind="ExternalInput").ap()
    v = nc.dram_tensor("v", [HEADS_PER_CORE, W, D], f32r, kind="ExternalInput").ap()
    maskT = nc.dram_tensor("maskT", [W, W], f32, kind="ExternalInput").ap()
    ones = nc.dram_tensor("ones", [128, 1], f32r, kind="ExternalInput").ap()
    outT = nc.dram_tensor("outT", [HEADS_PER_CORE, D, L], f32, kind="ExternalOutput").ap()

    masked_groups = max(1, W // QG)  # leading groups of each head needing the mask

    with tile.TileContext(nc) as tc:
        with ExitStack() as ctx:
            const = ctx.enter_context(tc.tile_pool(name="const", bufs=1))
            kt_pool = ctx.enter_context(tc.tile_pool(name="kt", bufs=2))
            v_pool = ctx.enter_context(tc.tile_pool(name="v", bufs=2))
            q_pool = ctx.enter_context(tc.tile_pool(name="q", bufs=2))
            o_pool = ctx.enter_context(tc.tile_pool(name="o", bufs=2))
            p_pool = ctx.enter_context(tc.tile_pool(name="p", bufs=3))
            recip_pool = ctx.enter_context(tc.tile_pool(name="recip", bufs=3))
            rbc_pool = ctx.enter_context(tc.tile_pool(name="rbc", bufs=3))
            s_psum = ctx.enter_context(tc.tile_pool(name="s_ps", bufs=1, space="PSUM"))
            o_psum = ctx.enter_context(tc.tile_pool(name="o_ps", bufs=2, space="PSUM"))
            sum_psum = ctx.enter_context(tc.tile_pool(name="sum_ps", bufs=2, space="PSUM"))

            # core-resident constants
            mask_t = const.tile([128, NCHUNK * W], f32, tag="mask")
            for c in range(NCHUNK):
                nc.gpsimd.dma_start(mask_t[:, c * W:(c + 1) * W],
                                    maskT[c * 128:(c + 1) * 128, :])
            ones_t = const.tile([128, 1], f32r, tag="ones")
            nc.gpsimd.dma_start(ones_t[:], ones[:])

            for h in range(HEADS_PER_CORE):
                kt_t = kt_pool.tile([128, W], f32r, tag="kt")
                nc.gpsimd.dma_start(kt_t[:], kT[h])
                v_t = v_pool.tile([128, NCHUNK * D], f32r, tag="v")
                for c in range(NCHUNK):
                    nc.gpsimd.dma_start(v_t[:, c * D:(c + 1) * D],
                                        v[h, c * 128:(c + 1) * 128, :])
                qt_t = q_pool.tile([128, L], f32r, tag="q")
                for i in range(4):
                    nc.gpsimd.dma_start(qt_t[:, i * (L // 4):(i + 1) * (L // 4)],
                                        qT[h, :, i * (L // 4):(i + 1) * (L // 4)])
                o_t = o_pool.tile([128, L], f32, tag="o")

                for g in range(NG):
                    qs = slice(g * QG, (g + 1) * QG)
                    s_ps = s_psum.tile([128, NCHUNK * QG], f32, tag="s")
                    for c in range(NCHUNK):
                        nc.tensor.matmul(
                            s_ps[:, c * QG:(c + 1) * QG],
                            lhsT=kt_t[:, c * 128:(c + 1) * 128],
                            rhs=qt_t[:, qs],
                            start=True, stop=True,
                        )
                    if g < masked_groups:
                        for c in range(NCHUNK):
                            if c * 128 + 127 > g * QG:
                                nc.vector.tensor_add(
                                    s_ps[:, c * QG:(c + 1) * QG],
                                    s_ps[:, c * QG:(c + 1) * QG],
                                    mask_t[:, c * W + g * QG:c * W + (g + 1) * QG],
                                )
                    p_t = p_pool.tile([128, NCHUNK * QG], f32r, tag="p")
                    nc.scalar.activation(p_t[:], s_ps[:],
                                         mybir.ActivationFunctionType.Exp,
                                         scale=SCALE)
                    sums_ps = sum_psum.tile([1, QG], f32, tag="sums_ps")
                    for c in range(NCHUNK):
                        nc.tensor.matmul(
                            sums_ps[:],
                            lhsT=ones_t[:],
                            rhs=p_t[:, c * QG:(c + 1) * QG],
                            start=(c == 0), stop=(c == NCHUNK - 1),
                        )
                    recip_t = recip_pool.tile([1, QG], f32, tag="recip")
                    nc.vector.reciprocal_approx_fast(recip_t[:], sums_ps[:])
                    rbc_t = rbc_pool.tile([128, QG], f32, tag="rbc")
                    nc.gpsimd.partition_broadcast(rbc_t[:], recip_t[:])
                    o_ps = o_psum.tile([128, QG], f32, tag="ops")
                    for c in range(NCHUNK):
                        nc.tensor.matmul(
                            o_ps[:],
                            lhsT=v_t[:, c * D:(c + 1) * D],
                            rhs=p_t[:, c * QG:(c + 1) * QG],
                            start=(c == 0), stop=(c == NCHUNK - 1),
                        )
                    nc.vector.tensor_mul(o_t[:, qs], o_ps[:], rbc_t[:])

                for i in range(4):
                    nc.gpsimd.dma_start(outT[h, :, i * (L // 4):(i + 1) * (L // 4)],
                                        o_t[:, i * (L // 4):(i + 1) * (L // 4)])

    nc.compile()
    return nc


def _get_compiled():
    global _COMPILED
    if _COMPILED is None:
        _COMPILED = _build()
    return _COMPILED


def _make_in_maps(query, keys, values):
    q = np.asarray(query, dtype=np.float32)
    k = np.asarray(keys, dtype=np.float32)
    v = np.asarray(values, dtype=np.float32)

    qf = q.reshape(B * H, L, D)
    kf = k.reshape(B * H, L, D)[:, L - W:, :]
    vf = v.reshape(B * H, L, D)[:, L - W:, :]

    # additive mask in S^T layout: maskT[j, i] = NEG where query i < slot j
    mT = np.where(np.arange(W)[None, :] < np.arange(W)[:, None],
                  np.float32(NEG), np.float32(0.0))
    ones = np.ones((128, 1), dtype=np.float32)

    in_maps = []
    for core in range(N_CORES):
        s = slice(core * HEADS_PER_CORE, (core + 1) * HEADS_PER_CORE)
        in_maps.append({
            "qT": np.ascontiguousarray(qf[s].transpose(0, 2, 1)),
            "kT": np.ascontiguousarray(kf[s].transpose(0, 2, 1)),
            "v": np.ascontiguousarray(vf[s]),
            "maskT": mT,
            "ones": ones,
        })
    return in_maps


def kernel(query, keys, values, window_size):
    from concourse.bass_utils import run_bass_kernel_spmd

    w = int(window_size)
    assert np.asarray(query).shape == (B, H, L, D) and w == W

    nc = _get_compiled()
    in_maps = _make_in_maps(query, keys, values)
    res = run_bass_kernel_spmd(nc, in_maps, core_ids=list(range(N_CORES)))
    outs = [res.results[c]["outT"].transpose(0, 2, 1) for c in range(N_CORES)]
    return np.concatenate(outs, axis=0).reshape(B, H, L, D)


# revision 9
# speedup vs baseline: 2.0070x; 2.0070x over previous
"""Sliding-window attention Trainium2 Bass kernel.

Problem: B=4, H=32, L=4096, D=128, window=512.
reference: attends over the LAST w=512 key/value positions; query row i may
only see window slot j when j <= i (slots are key positions L-w+j).

Sharding: B*H = 128 (b,h) pairs split across 8 cores -> 16 heads/core.
Pure data parallelism, no collectives.

Per-group (512 queries) on-device algorithm, all matmuls f32r (1 cycle/row):
  S^T chunks [wc=128, 512] = (K^T chunk)^T . (Q^T group)      (PE -> PSUM)
  mask-add on the first 512 queries                           (DVE)
  P^T = exp(S^T / sqrt(D))                                    (ACT, 2 ops/group)
  rowsum[1, 512] = ones^T @ P^T chunks (accumulated)          (PE -> PSUM)
  recip = approx 1/rowsum (~2^-18 rel err)                    (DVE)
  bcast recip to 128 partitions                               (GPSIMD)
  O^T [D, 512] += V_c^T @ P^T chunks                          (PE -> PSUM)
  out = O^T * recip_bcast                                     (DVE, PSUM->SBUF)

The emission is software-pipelined one group deep: PE sees
[S-matmuls(g); rowsum(g-1); PV(g-1)] so it never waits on the ACT exp and
stays HAM-warm. PSUM: two [128,1024] S half-tiles (x2 bufs, 4 banks) + one
[128,1024] O tile holding O in bank0 and the rowsums in bank1 (x2 bufs,
4 banks).

Host pre-transposes Q/K to [head, D, L] layout and post-transposes the
[head, D, L] output back to [B, H, L, D].
"""

import math
from contextlib import ExitStack

import numpy as np

N_CORES = 8
B, H, L, D = 4, 32, 4096, 128
W = 512            # window
HEADS_PER_CORE = (B * H) // N_CORES   # 16
QG = 512           # queries per group
NG = L // QG       # groups per head (8)
NCHUNK = W // 128  # 4 window chunks
NEG = -1.0e9       # additive mask value (pre-scale)
SCALE = 1.0 / math.sqrt(D)

_COMPILED = None


def _build():
    import concourse.tile as tile
    from concourse import bacc, mybir

    nc = bacc.Bacc("TRN2", target_bir_lowering=False, debug=False,
                   num_devices=N_CORES)

    f32r = mybir.dt.float32r
    f32 = mybir.dt.float32

    qT = nc.dram_tensor("qT", [HEADS_PER_CORE, D, L], f32r, kind="ExternalInput").ap()
    kT = nc.dram_tensor("kT", [HEADS_PER_CORE, D, W], f32r, kind="ExternalInput").ap()
    v = nc.dram_tensor("v", [HEADS_PER_CORE, W, D], f32r, kind="ExternalInput").ap()
    maskT = nc.dram_tensor("maskT", [W, W], f32, kind="ExternalInput").ap()
    ones = nc.dram_tensor("ones", [128, 1], f32r, kind="ExternalInput").ap()
    outT = nc.dram_tensor("outT", [HEADS_PER_CORE, D, L], f32, kind="ExternalOutput").ap()

    with tile.TileContext(nc) as tc:
        with ExitStack() as ctx:
            const = ctx.enter_context(tc.tile_pool(name="const", bufs=1))
            kt_pool = ctx.enter_context(tc.tile_pool(name="kt", bufs=2))
            v_pool = ctx.enter_context(tc.tile_pool(name="v", bufs=2))
            q_pool = ctx.enter_context(tc.tile_pool(name="q", bufs=2))
            o_pool = ctx.enter_context(tc.tile_pool(name="o", bufs=2))
            p_pool = ctx.enter_context(tc.tile_pool(name="p", bufs=3))
            recip_pool = ctx.enter_context(tc.tile_pool(name="recip", bufs=3))
            rbc_pool = ctx.enter_context(tc.tile_pool(name="rbc", bufs=3))
            s_psum = ctx.enter_context(tc.tile_pool(name="s_ps", bufs=2, space="PSUM"))
            o_psum = ctx.enter_context(tc.tile_pool(name="o_ps", bufs=2, space="PSUM"))

            mask_t = const.tile([128, NCHUNK * W], f32, tag="mask")
            for c in range(NCHUNK):
                nc.gpsimd.dma_start(mask_t[:, c * W:(c + 1) * W],
                                    maskT[c * 128:(c + 1) * 128, :])
            ones_t = const.tile([128, 1], f32r, tag="ones")
            nc.gpsimd.dma_start(ones_t[:], ones[:])

            head_tiles = {}

            def load_head(h):
                kt_t = kt_pool.tile([128, W], f32r, tag="kt")
                nc.gpsimd.dma_start(kt_t[:], kT[h])
                v_t = v_pool.tile([128, NCHUNK * D], f32r, tag="v")
                for c in range(NCHUNK):
                    nc.gpsimd.dma_start(v_t[:, c * D:(c + 1) * D],
                                        v[h, c * 128:(c + 1) * 128, :])
                qt_t = q_pool.tile([128, L], f32r, tag="q")
                for i in range(8):
                    nc.gpsimd.dma_start(qt_t[:, i * (L // 8):(i + 1) * (L // 8)],
                                        qT[h, :, i * (L // 8):(i + 1) * (L // 8)])
                o_t = o_pool.tile([128, L], f32, tag="o")
                head_tiles[h] = (kt_t, v_t, qt_t, o_t)

            def emit_front(h, g):
                """S matmuls + mask + exp for group (h, g). Returns stage state."""
                kt_t, v_t, qt_t, o_t = head_tiles[h]
                qs = slice(g * QG, (g + 1) * QG)
                halves = []
                for half in range(2):
                    s_ps = s_psum.tile([128, 2 * QG], f32, tag="s")
                    for ci in range(2):
                        c = half * 2 + ci
                        nc.tensor.matmul(
                            s_ps[:, ci * QG:(ci + 1) * QG],
                            lhsT=kt_t[:, c * 128:(c + 1) * 128],
                            rhs=qt_t[:, qs],
                            start=True, stop=True,
                        )
                    halves.append(s_ps)
                if g == 0:
                    for half in range(2):
                        for ci in range(2):
                            c = half * 2 + ci
                            nc.vector.tensor_add(
                                halves[half][:, ci * QG:(ci + 1) * QG],
                                halves[half][:, ci * QG:(ci + 1) * QG],
                                mask_t[:, c * W + g * QG:c * W + (g + 1) * QG],
                            )
                p_t = p_pool.tile([128, NCHUNK * QG], f32r, tag="p")
                for half in range(2):
                    nc.scalar.activation(
                        p_t[:, half * 2 * QG:(half + 1) * 2 * QG],
                        halves[half][:],
                        mybir.ActivationFunctionType.Exp, scale=SCALE)
                return (h, g, p_t)

            def emit_back(stage):
                """rowsum + recip + bcast + PV + normalize for a front stage."""
                h, g, p_t = stage
                kt_t, v_t, qt_t, o_t = head_tiles[h]
                qs = slice(g * QG, (g + 1) * QG)
                o_ps = o_psum.tile([128, 2 * QG], f32, tag="ops")
                sums = o_ps[0:1, QG:2 * QG]       # bank 1
                for c in range(NCHUNK):
                    nc.tensor.matmul(
                        sums,
                        lhsT=ones_t[:],
                        rhs=p_t[:, c * QG:(c + 1) * QG],
                        start=(c == 0), stop=(c == NCHUNK - 1),
                    )
                recip_t = recip_pool.tile([1, QG], f32, tag="recip")
                nc.vector.reciprocal_approx_fast(recip_t[:], sums)
                rbc_t = rbc_pool.tile([128, QG], f32, tag="rbc")
                nc.gpsimd.partition_broadcast(rbc_t[:], recip_t[:])
                for c in range(NCHUNK):
                    nc.tensor.matmul(
                        o_ps[:, 0:QG],            # bank 0
                        lhsT=v_t[:, c * D:(c + 1) * D],
                        rhs=p_t[:, c * QG:(c + 1) * QG],
                        start=(c == 0), stop=(c == NCHUNK - 1),
                    )
                nc.vector.tensor_mul(o_t[:, qs], o_ps[:, 0:QG], rbc_t[:])
                if g == NG - 1:
                    for i in range(4):
                        nc.gpsimd.dma_start(
                            outT[h, :, i * (L // 4):(i + 1) * (L // 4)],
                            o_t[:, i * (L // 4):(i + 1) * (L // 4)])
                    del head_tiles[h]

            prev = None
            for it in range(HEADS_PER_CORE * NG):
                h, g = divmod(it, NG)
                if g == 0:
                    load_head(h)
                cur = emit_front(h, g)
                if prev is not None:
                    emit_back(prev)
                prev = cur
            emit_back(prev)

    nc.compile()
    return nc


def _get_compiled():
    global _COMPILED
    if _COMPILED is None:
        _COMPILED = _build()
    return _COMPILED


def _make_in_maps(query, keys, values):
    q = np.asarray(query, dtype=np.float32)
    k = np.asarray(keys, dtype=np.float32)
    v = np.asarray(values, dtype=np.float32)

    qf = q.reshape(B * H, L, D)
    kf = k.reshape(B * H, L, D)[:, L - W:, :]
    vf = v.reshape(B * H, L, D)[:, L - W:, :]

    # additive mask in S^T layout: maskT[j, i] = NEG where query i < slot j
    mT = np.where(np.arange(W)[None, :] < np.arange(W)[:, None],
                  np.float32(NEG), np.float32(0.0))
    ones = np.ones((128, 1), dtype=np.float32)

    in_maps = []
    for core in range(N_CORES):
        s = slice(core * HEADS_PER_CORE, (core + 1) * HEADS_PER_CORE)
        in_maps.append({
            "qT": np.ascontiguousarray(qf[s].transpose(0, 2, 1)),
            "kT": np.ascontiguousarray(kf[s].transpose(0, 2, 1)),
            "v": np.ascontiguousarray(vf[s]),
            "maskT": mT,
            "ones": ones,
        })
    return in_maps


def kernel(query, keys, values, window_size):
    from concourse.bass_utils import run_bass_kernel_spmd

    w = int(window_size)
    assert np.asarray(query).shape == (B, H, L, D) and w == W

    nc = _get_compiled()
    in_maps = _make_in_maps(query, keys, values)
    res = run_bass_kernel_spmd(nc, in_maps, core_ids=list(range(N_CORES)))
    outs = [res.results[c]["outT"].transpose(0, 2, 1) for c in range(N_CORES)]
    return np.concatenate(outs, axis=0).reshape(B, H, L, D)


# revision 12
# speedup vs baseline: 2.1600x; 1.0762x over previous
"""Sliding-window attention Trainium2 Bass kernel.

Problem: B=4, H=32, L=4096, D=128, window=512.
reference: attends over the LAST w=512 key/value positions; query row i may
only see window slot j when j <= i (slots are key positions L-w+j).

Sharding: B*H = 128 (b,h) pairs split across 8 cores -> 16 heads/core.
Pure data parallelism, no collectives.

Per-group (512 queries) on-device algorithm, all matmuls f32r (1 cycle/row):
  S^T chunks [wc=128, 512] = (K^T chunk)^T . (Q^T group)      (PE -> PSUM)
  mask-add on the first 512 queries                           (DVE)
  P^T = exp(S^T / sqrt(D))                                    (ACT, 2 ops/group)
  rowsum[1, 512] = ones^T @ P^T chunks (accumulated)          (PE -> PSUM)
  recip = approx 1/rowsum (~2^-18 rel err)                    (DVE)
  bcast recip to 128 partitions                               (GPSIMD)
  O^T [D, 512] += V_c^T @ P^T chunks                          (PE -> PSUM)
  out = O^T * recip_bcast                                     (DVE, PSUM->SBUF)

The emission is software-pipelined one group deep: PE sees
[S-matmuls(g); rowsum(g-1); PV(g-1)] so it never waits on the ACT exp and
stays HAM-warm. PSUM: two [128,1024] S half-tiles (x2 bufs, 4 banks) + one
[128,1024] O tile holding O in bank0 and the rowsums in bank1 (x2 bufs,
4 banks).

Host pre-transposes Q/K to [head, D, L] layout and post-transposes the
[head, D, L] output back to [B, H, L, D].
"""

import math
from contextlib import ExitStack

import numpy as np

N_CORES = 8
B, H, L, D = 4, 32, 4096, 128
W = 512            # window
HEADS_PER_CORE = (B * H) // N_CORES   # 16
QG = 512           # queries per group
NG = L // QG       # groups per head (8)
NCHUNK = W // 128  # 4 window chunks
NEG = -1.0e9       # additive mask value (pre-scale)
SCALE = 1.0 / math.sqrt(D)

_COMPILED = None


def _build():
    import concourse.tile as tile
    from concourse import bacc, mybir

    nc = bacc.Bacc("TRN2", target_bir_lowering=False, debug=False,
                   num_devices=N_CORES)

    f32r = mybir.dt.float32r
    f32 = mybir.dt.float32

    qT = nc.dram_tensor("qT", [HEADS_PER_CORE, D, L], f32r, kind="ExternalInput").ap()
    kT = nc.dram_tensor("kT", [HEADS_PER_CORE, D, W], f32r, kind="ExternalInput").ap()
    v = nc.dram_tensor("v", [HEADS_PER_CORE, W, D], f32r, kind="ExternalInput").ap()
    maskT = nc.dram_tensor("maskT", [W, W], f32, kind="ExternalInput").ap()
    ones = nc.dram_tensor("ones", [128, 1], f32r, kind="ExternalInput").ap()
    outT = nc.dram_tensor("outT", [HEADS_PER_CORE, D, L], f32, kind="ExternalOutput").ap()

    with tile.TileContext(nc) as tc:
        with ExitStack() as ctx:
            const = ctx.enter_context(tc.tile_pool(name="const", bufs=1))
            kt_pool = ctx.enter_context(tc.tile_pool(name="kt", bufs=2))
            v_pool = ctx.enter_context(tc.tile_pool(name="v", bufs=2))
            q_pool = ctx.enter_context(tc.tile_pool(name="q", bufs=2))
            o_pool = ctx.enter_context(tc.tile_pool(name="o", bufs=2))
            p_pool = ctx.enter_context(tc.tile_pool(name="p", bufs=3))
            recip_pool = ctx.enter_context(tc.tile_pool(name="recip", bufs=3))
            rbc_pool = ctx.enter_context(tc.tile_pool(name="rbc", bufs=3))
            s_psum = ctx.enter_context(tc.tile_pool(name="s_ps", bufs=2, space="PSUM"))
            o_psum = ctx.enter_context(tc.tile_pool(name="o_ps", bufs=2, space="PSUM"))

            mask_t = const.tile([128, NCHUNK * W], f32, tag="mask")
            for c in range(NCHUNK):
                nc.gpsimd.dma_start(mask_t[:, c * W:(c + 1) * W],
                                    maskT[c * 128:(c + 1) * 128, :])
            ones_t = const.tile([128, 1], f32r, tag="ones")
            nc.gpsimd.dma_start(ones_t[:], ones[:])

            head_tiles = {}

            def load_head(h):
                kt_t = kt_pool.tile([128, W], f32r, tag="kt")
                nc.sync.dma_start(kt_t[:], kT[h])
                v_t = v_pool.tile([128, NCHUNK * D], f32r, tag="v")
                for c in range(NCHUNK):
                    nc.sync.dma_start(v_t[:, c * D:(c + 1) * D],
                                      v[h, c * 128:(c + 1) * 128, :])
                qt_t = q_pool.tile([128, L], f32r, tag="q")
                for i in range(8):
                    nc.sync.dma_start(qt_t[:, i * (L // 8):(i + 1) * (L // 8)],
                                      qT[h, :, i * (L // 8):(i + 1) * (L // 8)])
                o_t = o_pool.tile([128, L], f32, tag="o")
                head_tiles[h] = (kt_t, v_t, qt_t, o_t)

            def emit_front(h, g):
                """S matmuls + mask + exp for group (h, g). Returns stage state."""
                kt_t, v_t, qt_t, o_t = head_tiles[h]
                qs = slice(g * QG, (g + 1) * QG)
                halves = []
                for half in range(2):
                    s_ps = s_psum.tile([128, 2 * QG], f32, tag="s")
                    for ci in range(2):
                        c = half * 2 + ci
                        # Group 0: queries < c*128 can't see chunk c — the mask
                        # add puts -1e9 over the skipped (stale, bounded) psum
                        # region, so exp() zeroes it. Shrink the matmul. Not at
                        # h==0 where the stale psum could be inf/nan garbage.
                        q_lo = c * 128 if (g == 0 and h > 0) else 0
                        nc.tensor.matmul(
                            s_ps[:, ci * QG + q_lo:(ci + 1) * QG],
                            lhsT=kt_t[:, c * 128:(c + 1) * 128],
                            rhs=qt_t[:, g * QG + q_lo:(g + 1) * QG],
                            start=True, stop=True,
                        )
                    halves.append(s_ps)
                if g == 0:
                    for half in range(2):
                        for ci in range(2):
                            c = half * 2 + ci
                            nc.vector.tensor_add(
                                halves[half][:, ci * QG:(ci + 1) * QG],
                                halves[half][:, ci * QG:(ci + 1) * QG],
                                mask_t[:, c * W + g * QG:c * W + (g + 1) * QG],
                            )
                p_t = p_pool.tile([128, NCHUNK * QG], f32r, tag="p")
                for half in range(2):
                    nc.scalar.activation(
                        p_t[:, half * 2 * QG:(half + 1) * 2 * QG],
                        halves[half][:],
                        mybir.ActivationFunctionType.Exp, scale=SCALE)
                return (h, g, p_t)

            def emit_back(stage):
                """rowsum + recip + bcast + PV + normalize for a front stage."""
                h, g, p_t = stage
                kt_t, v_t, qt_t, o_t = head_tiles[h]
                qs = slice(g * QG, (g + 1) * QG)
                o_ps = o_psum.tile([128, 2 * QG], f32, tag="ops")
                sums = o_ps[0:1, QG:2 * QG]       # bank 1
                for c in range(NCHUNK):
                    nc.tensor.matmul(
                        sums,
                        lhsT=ones_t[:],
                        rhs=p_t[:, c * QG:(c + 1) * QG],
                        start=(c == 0), stop=(c == NCHUNK - 1),
                    )
                recip_t = recip_pool.tile([1, QG], f32, tag="recip")
                nc.vector.reciprocal_approx_fast(recip_t[:], sums)
                rbc_t = rbc_pool.tile([128, QG], f32, tag="rbc")
                nc.gpsimd.partition_broadcast(rbc_t[:], recip_t[:])
                for c in range(NCHUNK):
                    nc.tensor.matmul(
                        o_ps[:, 0:QG],            # bank 0
                        lhsT=v_t[:, c * D:(c + 1) * D],
                        rhs=p_t[:, c * QG:(c + 1) * QG],
                        start=(c == 0), stop=(c == NCHUNK - 1),
                    )
                nc.vector.tensor_mul(o_t[:, qs], o_ps[:, 0:QG], rbc_t[:])
                if g == NG - 1:
                    for i in range(4):
                        nc.gpsimd.dma_start(
                            outT[h, :, i * (L // 4):(i + 1) * (L // 4)],
                            o_t[:, i * (L // 4):(i + 1) * (L // 4)])
                    del head_tiles[h]

            prev = None
            for it in range(HEADS_PER_CORE * NG):
                h, g = divmod(it, NG)
                if g == 0:
                    load_head(h)
                cur = emit_front(h, g)
                if prev is not None:
                    emit_back(prev)
                prev = cur
            emit_back(prev)

    nc.compile()
    return nc


def _get_compiled():
    global _COMPILED
    if _COMPILED is None:
        _COMPILED = _build()
    return _COMPILED


def _make_in_maps(query, keys, values):
    q = np.asarray(query, dtype=np.float32)
    k = np.asarray(keys, dtype=np.float32)
    v = np.asarray(values, dtype=np.float32)

    qf = q.reshape(B * H, L, D)
    kf = k.reshape(B * H, L, D)[:, L - W:, :]
    vf = v.reshape(B * H, L, D)[:, L - W:, :]

    # additive mask in S^T layout: maskT[j, i] = NEG where query i < slot j
    mT = np.where(np.arange(W)[None, :] < np.arange(W)[:, None],
                  np.float32(NEG), np.float32(0.0))
    ones = np.ones((128, 1), dtype=np.float32)

    in_maps = []
    for core in range(N_CORES):
        s = slice(core * HEADS_PER_CORE, (core + 1) * HEADS_PER_CORE)
        in_maps.append({
            "qT": np.ascontiguousarray(qf[s].transpose(0, 2, 1)),
            "kT": np.ascontiguousarray(kf[s].transpose(0, 2, 1)),
            "v": np.ascontiguousarray(vf[s]),
            "maskT": mT,
            "ones": ones,
        })
    return in_maps


def kernel(query, keys, values, window_size):
    from concourse.bass_utils import run_bass_kernel_spmd

    w = int(window_size)
    assert np.asarray(query).shape == (B, H, L, D) and w == W

    nc = _get_compiled()
    in_maps = _make_in_maps(query, keys, values)
    res = run_bass_kernel_spmd(nc, in_maps, core_ids=list(range(N_CORES)))
    outs = [res.results[c]["outT"].transpose(0, 2, 1) for c in range(N_CORES)]
    return np.concatenate(outs, axis=0).reshape(B, H, L, D)


# revision 13
# speedup vs baseline: 2.1612x; 1.0006x over previous
"""Sliding-window attention Trainium2 Bass kernel.

Problem: B=4, H=32, L=4096, D=128, window=512.
reference: attends over the LAST w=512 key/value positions; query row i may
only see window slot j when j <= i (slots are key positions L-w+j).

Sharding: B*H = 128 (b,h) pairs split across 8 cores -> 16 heads/core.
Pure data parallelism, no collectives.

Per-group (512 queries) on-device algorithm, all matmuls f32r (1 cycle/row):
  S^T chunks [wc=128, 512] = (K^T chunk)^T . (Q^T group)      (PE -> PSUM)
  mask-add on the first 512 queries                           (DVE)
  P^T = exp(S^T / sqrt(D))                                    (ACT, 2 ops/group)
  rowsum[1, 512] = ones^T @ P^T chunks (accumulated)          (PE -> PSUM)
  recip = approx 1/rowsum (~2^-18 rel err)                    (DVE)
  bcast recip to 128 partitions                               (GPSIMD)
  O^T [D, 512] += V_c^T @ P^T chunks                          (PE -> PSUM)
  out = O^T * recip_bcast                                     (DVE, PSUM->SBUF)

The emission is software-pipelined one group deep: PE sees
[S-matmuls(g); rowsum(g-1); PV(g-1)] so it never waits on the ACT exp and
stays HAM-warm. PSUM: two [128,1024] S half-tiles (x2 bufs, 4 banks) + one
[128,1024] O tile holding O in bank0 and the rowsums in bank1 (x2 bufs,
4 banks).

Host pre-transposes Q/K to [head, D, L] layout and post-transposes the
[head, D, L] output back to [B, H, L, D].
"""

import math
from contextlib import ExitStack

import numpy as np

N_CORES = 8
B, H, L, D = 4, 32, 4096, 128
W = 512            # window
HEADS_PER_CORE = (B * H) // N_CORES   # 16
QG = 512           # queries per group
NG = L // QG       # groups per head (8)
NCHUNK = W // 128  # 4 window chunks
NEG = -1.0e9       # additive mask value (pre-scale)
SCALE = 1.0 / math.sqrt(D)

_COMPILED = None


def _build():
    import concourse.tile as tile
    from concourse import bacc, mybir

    nc = bacc.Bacc("TRN2", target_bir_lowering=False, debug=False,
                   num_devices=N_CORES)

    f32r = mybir.dt.float32r
    f32 = mybir.dt.float32

    qT = nc.dram_tensor("qT", [HEADS_PER_CORE, D, L], f32r, kind="ExternalInput").ap()
    kT = nc.dram_tensor("kT", [HEADS_PER_CORE, D, W], f32r, kind="ExternalInput").ap()
    v = nc.dram_tensor("v", [HEADS_PER_CORE, W, D], f32r, kind="ExternalInput").ap()
    maskT = nc.dram_tensor("maskT", [W, W], f32, kind="ExternalInput").ap()
    ones = nc.dram_tensor("ones", [128, 1], f32r, kind="ExternalInput").ap()
    outT = nc.dram_tensor("outT", [HEADS_PER_CORE, D, L], f32, kind="ExternalOutput").ap()

    with tile.TileContext(nc) as tc:
        with ExitStack() as ctx:
            const = ctx.enter_context(tc.tile_pool(name="const", bufs=1))
            kt_pool = ctx.enter_context(tc.tile_pool(name="kt", bufs=2))
            v_pool = ctx.enter_context(tc.tile_pool(name="v", bufs=2))
            q_pool = ctx.enter_context(tc.tile_pool(name="q", bufs=2))
            o_pool = ctx.enter_context(tc.tile_pool(name="o", bufs=2))
            p_pool = ctx.enter_context(tc.tile_pool(name="p", bufs=3))
            recip_pool = ctx.enter_context(tc.tile_pool(name="recip", bufs=3))
            rbc_pool = ctx.enter_context(tc.tile_pool(name="rbc", bufs=3))
            s_psum = ctx.enter_context(tc.tile_pool(name="s_ps", bufs=2, space="PSUM"))
            o_psum = ctx.enter_context(tc.tile_pool(name="o_ps", bufs=2, space="PSUM"))

            mask_t = const.tile([128, NCHUNK * W], f32, tag="mask")
            for c in range(NCHUNK):
                nc.gpsimd.dma_start(mask_t[:, c * W:(c + 1) * W],
                                    maskT[c * 128:(c + 1) * 128, :])
            ones_t = const.tile([128, 1], f32r, tag="ones")
            nc.gpsimd.dma_start(ones_t[:], ones[:])

            head_tiles = {}

            def load_head(h):
                kt_t = kt_pool.tile([128, W], f32r, tag="kt")
                nc.sync.dma_start(kt_t[:], kT[h])
                v_t = v_pool.tile([128, NCHUNK * D], f32r, tag="v")
                for c in range(NCHUNK):
                    nc.sync.dma_start(v_t[:, c * D:(c + 1) * D],
                                      v[h, c * 128:(c + 1) * 128, :])
                qt_t = q_pool.tile([128, L], f32r, tag="q")
                for i in range(8):
                    nc.sync.dma_start(qt_t[:, i * (L // 8):(i + 1) * (L // 8)],
                                      qT[h, :, i * (L // 8):(i + 1) * (L // 8)])
                o_t = o_pool.tile([128, L], f32, tag="o")
                head_tiles[h] = (kt_t, v_t, qt_t, o_t)

            def emit_front(h, g):
                """S matmuls + mask + exp for group (h, g). Returns stage state."""
                kt_t, v_t, qt_t, o_t = head_tiles[h]
                qs = slice(g * QG, (g + 1) * QG)
                halves = []
                for half in range(2):
                    s_ps = s_psum.tile([128, 2 * QG], f32, tag="s")
                    for ci in range(2):
                        c = half * 2 + ci
                        # Group 0: queries < c*128 can't see chunk c — the mask
                        # add puts -1e9 over the skipped (stale, bounded) psum
                        # region, so exp() zeroes it. Shrink the matmul. Not at
                        # h==0 where the stale psum could be inf/nan garbage.
                        q_lo = c * 128 if (g == 0 and h > 0) else 0
                        nc.tensor.matmul(
                            s_ps[:, ci * QG + q_lo:(ci + 1) * QG],
                            lhsT=kt_t[:, c * 128:(c + 1) * 128],
                            rhs=qt_t[:, g * QG + q_lo:(g + 1) * QG],
                            start=True, stop=True,
                        )
                    halves.append(s_ps)
                if g == 0:
                    for half in range(2):
                        for ci in range(2):
                            c = half * 2 + ci
                            nc.vector.tensor_add(
                                halves[half][:, ci * QG:(ci + 1) * QG],
                                halves[half][:, ci * QG:(ci + 1) * QG],
                                mask_t[:, c * W + g * QG:c * W + (g + 1) * QG],
                            )
                p_t = p_pool.tile([128, NCHUNK * QG], f32r, tag="p")
                for half in range(2):
                    nc.scalar.activation(
                        p_t[:, half * 2 * QG:(half + 1) * 2 * QG],
                        halves[half][:],
                        mybir.ActivationFunctionType.Exp, scale=SCALE)
                return (h, g, p_t)

            def emit_back(stage):
                """rowsum + recip + bcast + PV + normalize for a front stage."""
                h, g, p_t = stage
                kt_t, v_t, qt_t, o_t = head_tiles[h]
                qs = slice(g * QG, (g + 1) * QG)
                o_ps = o_psum.tile([128, 2 * QG], f32, tag="ops")
                sums = o_ps[0:1, QG:2 * QG]       # bank 1
                for c in range(NCHUNK):
                    nc.tensor.matmul(
                        sums,
                        lhsT=ones_t[:],
                        rhs=p_t[:, c * QG:(c + 1) * QG],
                        start=(c == 0), stop=(c == NCHUNK - 1),
                    )
                recip_t = recip_pool.tile([1, QG], f32, tag="recip")
                nc.vector.reciprocal_approx_fast(recip_t[:], sums)
                rbc_t = rbc_pool.tile([128, QG], f32, tag="rbc")
                nc.gpsimd.partition_broadcast(rbc_t[:], recip_t[:])
                for c in range(NCHUNK):
                    nc.tensor.matmul(
                        o_ps[:, 0:QG],            # bank 0
                        lhsT=v_t[:, c * D:(c + 1) * D],
                        rhs=p_t[:, c * QG:(c + 1) * QG],
                        start=(c == 0), stop=(c == NCHUNK - 1),
                    )
                nc.vector.tensor_mul(o_t[:, qs], o_ps[:, 0:QG], rbc_t[:])
                if g == NG - 1:
                    for i in range(4):
                        nc.gpsimd.dma_start(
                            outT[h, :, i * (L // 4):(i + 1) * (L // 4)],
                            o_t[:, i * (L // 4):(i + 1) * (L // 4)])
                    del head_tiles[h]

            prev = None
            load_head(0)
            for it in range(HEADS_PER_CORE * NG):
                h, g = divmod(it, NG)
                if g == NG // 2 and h + 1 < HEADS_PER_CORE:
                    load_head(h + 1)   # prefetch next head during this one
                cur = emit_front(h, g)
                if prev is not None:
                    emit_back(prev)
                prev = cur
            emit_back(prev)

    nc.compile()
    return nc


def _get_compiled():
    global _COMPILED
    if _COMPILED is None:
        _COMPILED = _build()
    return _COMPILED


def _make_in_maps(query, keys, values):
    q = np.asarray(query, dtype=np.float32)
    k = np.asarray(keys, dtype=np.float32)
    v = np.asarray(values, dtype=np.float32)

    qf = q.reshape(B * H, L, D)
    kf = k.reshape(B * H, L, D)[:, L - W:, :]
    vf = v.reshape(B * H, L, D)[:, L - W:, :]

    # additive mask in S^T layout: maskT[j, i] = NEG where query i < slot j
    mT = np.where(np.arange(W)[None, :] < np.arange(W)[:, None],
                  np.float32(NEG), np.float32(0.0))
    ones = np.ones((128, 1), dtype=np.float32)

    in_maps = []
    for core in range(N_CORES):
        s = slice(core * HEADS_PER_CORE, (core + 1) * HEADS_PER_CORE)
        in_maps.append({
            "qT": np.ascontiguousarray(qf[s].transpose(0, 2, 1)),
            "kT": np.ascontiguousarray(kf[s].transpose(0, 2, 1)),
            "v": np.ascontiguousarray(vf[s]),
            "maskT": mT,
            "ones": ones,
        })
    return in_maps


def kernel(query, keys, values, window_size):
    from concourse.bass_utils import run_bass_kernel_spmd

    w = int(window_size)
    assert np.asarray(query).shape == (B, H, L, D) and w == W

    nc = _get_compiled()
    in_maps = _make_in_maps(query, keys, values)
    res = run_bass_kernel_spmd(nc, in_maps, core_ids=list(range(N_CORES)))
    outs = [res.results[c]["outT"].transpose(0, 2, 1) for c in range(N_CORES)]
    return np.concatenate(outs, axis=0).reshape(B, H, L, D)


# revision 15
# speedup vs baseline: 2.2562x; 1.0440x over previous
"""Sliding-window attention Trainium2 Bass kernel.

Problem: B=4, H=32, L=4096, D=128, window=512.
reference: attends over the LAST w=512 key/value positions; query row i may
only see window slot j when j <= i (slots are key positions L-w+j).

Sharding: B*H = 128 (b,h) pairs split across 8 cores -> 16 heads/core.
Pure data parallelism, no collectives.

Per-group (512 queries) on-device algorithm, all matmuls f32r (1 cycle/row):
  S^T chunks [wc=128, 512] = (K^T chunk)^T . (Q^T group)      (PE -> PSUM)
  mask-add on the first 512 queries                           (DVE)
  P^T = exp(S^T / sqrt(D))                                    (ACT, 2 ops/group)
  rowsum[1, 512] = ones^T @ P^T chunks (accumulated)          (PE -> PSUM)
  recip = approx 1/rowsum (~2^-18 rel err)                    (DVE)
  bcast recip to 128 partitions                               (GPSIMD)
  O^T [D, 512] += V_c^T @ P^T chunks                          (PE -> PSUM)
  out = O^T * recip_bcast                                     (DVE, PSUM->SBUF)

The emission is software-pipelined one group deep: PE sees
[S-matmuls(g); rowsum(g-1); PV(g-1)] so it never waits on the ACT exp and
stays HAM-warm. PSUM: two [128,1024] S half-tiles (x2 bufs, 4 banks) + one
[128,1024] O tile holding O in bank0 and the rowsums in bank1 (x2 bufs,
4 banks).

Host pre-transposes Q/K to [head, D, L] layout and post-transposes the
[head, D, L] output back to [B, H, L, D].
"""

import math
from contextlib import ExitStack

import numpy as np

N_CORES = 8
B, H, L, D = 4, 32, 4096, 128
W = 512            # window
HEADS_PER_CORE = (B * H) // N_CORES   # 16
QG = 512           # queries per group
NG = L // QG       # groups per head (8)
NCHUNK = W // 128  # 4 window chunks
NEG = -1.0e9       # additive mask value (pre-scale)
SCALE = 1.0 / math.sqrt(D)

_COMPILED = None


def _build():
    import concourse.tile as tile
    from concourse import bacc, mybir

    nc = bacc.Bacc("TRN2", target_bir_lowering=False, debug=False,
                   num_devices=N_CORES)

    f32r = mybir.dt.float32r
    f32 = mybir.dt.float32

    qT = nc.dram_tensor("qT", [HEADS_PER_CORE, D, L], f32r, kind="ExternalInput").ap()
    kT = nc.dram_tensor("kT", [HEADS_PER_CORE, D, W], f32r, kind="ExternalInput").ap()
    v = nc.dram_tensor("v", [HEADS_PER_CORE, W, D], f32r, kind="ExternalInput").ap()
    maskT = nc.dram_tensor("maskT", [W, W], f32, kind="ExternalInput").ap()
    ones = nc.dram_tensor("ones", [128, 1], f32r, kind="ExternalInput").ap()
    outT = nc.dram_tensor("outT", [HEADS_PER_CORE, D, L], f32, kind="ExternalOutput").ap()

    with tile.TileContext(nc) as tc:
        with ExitStack() as ctx:
            const = ctx.enter_context(tc.tile_pool(name="const", bufs=1))
            kt_pool = ctx.enter_context(tc.tile_pool(name="kt", bufs=2))
            v_pool = ctx.enter_context(tc.tile_pool(name="v", bufs=2))
            q_pool = ctx.enter_context(tc.tile_pool(name="q", bufs=2))
            o_pool = ctx.enter_context(tc.tile_pool(name="o", bufs=2))
            p_pool = ctx.enter_context(tc.tile_pool(name="p", bufs=3))
            recip_pool = ctx.enter_context(tc.tile_pool(name="recip", bufs=3))
            rbc_pool = ctx.enter_context(tc.tile_pool(name="rbc", bufs=3))
            s_psum = ctx.enter_context(tc.tile_pool(name="s_ps", bufs=2, space="PSUM"))
            o_psum = ctx.enter_context(tc.tile_pool(name="o_ps", bufs=2, space="PSUM"))

            mask_t = const.tile([128, NCHUNK * W], f32, tag="mask")
            for c in range(NCHUNK):
                nc.gpsimd.dma_start(mask_t[:, c * W:(c + 1) * W],
                                    maskT[c * 128:(c + 1) * 128, :])
            ones_t = const.tile([128, 1], f32r, tag="ones")
            nc.gpsimd.dma_start(ones_t[:], ones[:])

            head_tiles = {}

            def load_head(h):
                kt_t = kt_pool.tile([128, W], f32r, tag="kt")
                nc.sync.dma_start(kt_t[:], kT[h])
                v_t = v_pool.tile([128, NCHUNK * D], f32r, tag="v")
                for c in range(NCHUNK):
                    nc.sync.dma_start(v_t[:, c * D:(c + 1) * D],
                                      v[h, c * 128:(c + 1) * 128, :])
                qt_t = q_pool.tile([128, L], f32r, tag="q")
                for i in range(8):
                    nc.sync.dma_start(qt_t[:, i * (L // 8):(i + 1) * (L // 8)],
                                      qT[h, :, i * (L // 8):(i + 1) * (L // 8)])
                o_t = o_pool.tile([128, L], f32, tag="o")
                head_tiles[h] = (kt_t, v_t, qt_t, o_t)

            def emit_front(h, g):
                """S matmuls + mask + exp for group (h, g). Returns stage state."""
                kt_t, v_t, qt_t, o_t = head_tiles[h]
                qs = slice(g * QG, (g + 1) * QG)
                halves = []
                for half in range(2):
                    s_ps = s_psum.tile([128, 2 * QG], f32, tag="s")
                    for ci in range(2):
                        c = half * 2 + ci
                        # Group 0: queries < c*128 can't see chunk c — the mask
                        # add puts -1e9 over the skipped (stale, bounded) psum
                        # region, so exp() zeroes it. Shrink the matmul. Not at
                        # h==0 where the stale psum could be inf/nan garbage.
                        q_lo = c * 128 if (g == 0 and h > 0) else 0
                        nc.tensor.matmul(
                            s_ps[:, ci * QG + q_lo:(ci + 1) * QG],
                            lhsT=kt_t[:, c * 128:(c + 1) * 128],
                            rhs=qt_t[:, g * QG + q_lo:(g + 1) * QG],
                            start=True, stop=True,
                        )
                    halves.append(s_ps)
                if g == 0:
                    # only the diagonal 128x128 block of each chunk is
                    # partially masked; fully-masked rectangles (q < c*128)
                    # are skipped by the rowsum/PV matmuls instead.
                    for half in range(2):
                        for ci in range(2):
                            c = half * 2 + ci
                            blk = slice(ci * QG + c * 128, ci * QG + (c + 1) * 128)
                            mblk = slice(c * W + c * 128, c * W + (c + 1) * 128)
                            nc.vector.tensor_add(
                                halves[half][:, blk],
                                halves[half][:, blk],
                                mask_t[:, mblk],
                            )
                p_t = p_pool.tile([128, NCHUNK * QG], f32r, tag="p")
                for half in range(2):
                    nc.scalar.activation(
                        p_t[:, half * 2 * QG:(half + 1) * 2 * QG],
                        halves[half][:],
                        mybir.ActivationFunctionType.Exp, scale=SCALE)
                return (h, g, p_t)

            def emit_back(stage):
                """rowsum + recip + bcast + PV + normalize for a front stage."""
                h, g, p_t = stage
                kt_t, v_t, qt_t, o_t = head_tiles[h]
                qs = slice(g * QG, (g + 1) * QG)
                o_ps = o_psum.tile([128, 2 * QG], f32, tag="ops")
                sums = o_ps[0:1, QG:2 * QG]       # bank 1
                for c in range(NCHUNK):
                    # group 0: chunk c contributes nothing to queries < c*128
                    q_lo = c * 128 if g == 0 else 0
                    nc.tensor.matmul(
                        o_ps[0:1, QG + q_lo:2 * QG],
                        lhsT=ones_t[:],
                        rhs=p_t[:, c * QG + q_lo:(c + 1) * QG],
                        start=(c == 0), stop=(c == NCHUNK - 1),
                    )
                recip_t = recip_pool.tile([1, QG], f32, tag="recip")
                nc.vector.reciprocal_approx_fast(recip_t[:], sums)
                rbc_t = rbc_pool.tile([128, QG], f32, tag="rbc")
                nc.gpsimd.partition_broadcast(rbc_t[:], recip_t[:])
                for c in range(NCHUNK):
                    q_lo = c * 128 if g == 0 else 0
                    nc.tensor.matmul(
                        o_ps[:, q_lo:QG],         # bank 0
                        lhsT=v_t[:, c * D:(c + 1) * D],
                        rhs=p_t[:, c * QG + q_lo:(c + 1) * QG],
                        start=(c == 0), stop=(c == NCHUNK - 1),
                    )
                nc.vector.tensor_mul(o_t[:, qs], o_ps[:, 0:QG], rbc_t[:])
                if g == NG - 1:
                    for i in range(4):
                        nc.gpsimd.dma_start(
                            outT[h, :, i * (L // 4):(i + 1) * (L // 4)],
                            o_t[:, i * (L // 4):(i + 1) * (L // 4)])
                    del head_tiles[h]

            prev = None
            load_head(0)
            for it in range(HEADS_PER_CORE * NG):
                h, g = divmod(it, NG)
                if g == NG // 2 and h + 1 < HEADS_PER_CORE:
                    load_head(h + 1)   # prefetch next head during this one
                cur = emit_front(h, g)
                if prev is not None:
                    emit_back(prev)
                prev = cur
            emit_back(prev)

    nc.compile()
    return nc


def _get_compiled():
    global _COMPILED
    if _COMPILED is None:
        _COMPILED = _build()
    return _COMPILED


def _make_in_maps(query, keys, values):
    q = np.asarray(query, dtype=np.float32)
    k = np.asarray(keys, dtype=np.float32)
    v = np.asarray(values, dtype=np.float32)

    qf = q.reshape(B * H, L, D)
    kf = k.reshape(B * H, L, D)[:, L - W:, :]
    vf = v.reshape(B * H, L, D)[:, L - W:, :]

    # additive mask in S^T layout: maskT[j, i] = NEG where query i < slot j
    mT = np.where(np.arange(W)[None, :] < np.arange(W)[:, None],
                  np.float32(NEG), np.float32(0.0))
    ones = np.ones((128, 1), dtype=np.float32)

    in_maps = []
    for core in range(N_CORES):
        s = slice(core * HEADS_PER_CORE, (core + 1) * HEADS_PER_CORE)
        in_maps.append({
            "qT": np.ascontiguousarray(qf[s].transpose(0, 2, 1)),
            "kT": np.ascontiguousarray(kf[s].transpose(0, 2, 1)),
            "v": np.ascontiguousarray(vf[s]),
            "maskT": mT,
            "ones": ones,
        })
    return in_maps


def kernel(query, keys, values, window_size):
    from concourse.bass_utils import run_bass_kernel_spmd

    w = int(window_size)
    assert np.asarray(query).shape == (B, H, L, D) and w == W

    nc = _get_compiled()
    in_maps = _make_in_maps(query, keys, values)
    res = run_bass_kernel_spmd(nc, in_maps, core_ids=list(range(N_CORES)))
    outs = [res.results[c]["outT"].transpose(0, 2, 1) for c in range(N_CORES)]
    return np.concatenate(outs, axis=0).reshape(B, H, L, D)
